# Initial kernel scaffold
#
"""ContactMapLinear Trainium2 kernel.

res = tril((X @ P) @ (Q @ X^T), k=-1), X = features[0, 1:4097, :], 8-core SPMD.

Sharding (sequence-parallel): core c owns seq rows [512c, 512c+512).
  Phase A: AT_c = P^T @ X_c^T        [1024, 512]   (A = X@P, stored transposed)
  Phase B: B_c  = Q   @ X_c^T        [1024, 512]   (cols of B = Q @ X^T)
  AllGather B_c over 8 cores -> B    [1024, 4096]
  Phase C: S_c  = AT_c^T @ B         [512, 4096], masked on host (tril, k=-1)

All matmul inputs are bf16 (fp32 PSUM accumulation); output is fp32.
"""

import sys

import ml_dtypes
import numpy as np

_TRN_REPO = "/opt/trn_rl_repo"
if _TRN_REPO not in sys.path:
    sys.path.insert(0, _TRN_REPO)

D = 4096          # seq length / feature dim
I = 1024          # inner dim
N_CORES = 8
R = D // N_CORES  # 512 seq rows per core
P = 128           # partitions
KT = D // P       # 32 feature k-tiles
IT = I // P       # 8 inner tiles
BF16 = ml_dtypes.bfloat16

_CACHE = {}


def _build():
    import concourse.bass as bass
    import concourse.mybir as mybir
    import concourse.tile as tile
    from concourse import bacc

    dt = mybir.dt
    nc = bacc.Bacc("TRN2", target_bir_lowering=False, debug=False,
                   num_devices=N_CORES)

    xt_in = nc.declare_dram_parameter("xt", [D, R], dt.bfloat16, isOutput=False)
    p_in = nc.declare_dram_parameter("p", [D, I], dt.bfloat16, isOutput=False)
    qt_in = nc.declare_dram_parameter("qt", [D, I], dt.bfloat16, isOutput=False)
    out = nc.declare_dram_parameter("out", [R, D], dt.float32, isOutput=True)

    xt_ap = xt_in.ap().rearrange("(ko ki) n -> ki ko n", ki=P)   # [128, 32, 512]
    p_ap = p_in.ap().rearrange("(ko ki) m -> ki ko m", ki=P)     # [128, 32, 1024]
    qt_ap = qt_in.ap().rearrange("(ko ki) m -> ki ko m", ki=P)
    out_ap = out.ap().rearrange("(mo mi) n -> mi mo n", mi=P)    # [128, 4, 4096]

    with tile.TileContext(nc) as tc:
        with (
            tc.tile_pool(name="xt", bufs=KT) as xt_pool,
            tc.tile_pool(name="w", bufs=3) as w_pool,
            tc.tile_pool(name="ab", bufs=1) as ab_pool,
            tc.tile_pool(name="bj", bufs=2) as bj_pool,
            tc.tile_pool(name="oc", bufs=3) as oc_pool,
            tc.tile_pool(name="ps", bufs=8, space="PSUM") as ps_pool,
            tc.tile_pool(name="dram", bufs=1, space="DRAM") as dram_pool,
        ):
            # xt tiles: one per feature k-tile so compute can start per-tile
            xt_sb = []
            for k in range(KT):
                t = xt_pool.tile([P, R], dt.bfloat16, tag=f"xt{k}")
                nc.sync.dma_start(out=t[:], in_=xt_ap[:, k, :])
                xt_sb.append(t)

            at_sb = ab_pool.tile([P, IT, R], dt.bfloat16, tag="at")
            b_sb = ab_pool.tile([P, IT, R], dt.bfloat16, tag="b")

            # Phases A and B: k-outer streaming of weights, 8 PSUM banks
            # (one per inner m-tile) accumulate across all 32 k-tiles.
            for w_ap, dst in ((p_ap, at_sb), (qt_ap, b_sb)):
                psums = [ps_pool.tile([P, R], dt.float32) for _ in range(IT)]
                for k in range(KT):
                    w_sb = w_pool.tile([P, I], dt.bfloat16, tag="w")
                    nc.sync.dma_start(out=w_sb[:], in_=w_ap[:, k, :])
                    for m in range(IT):
                        nc.tensor.matmul(
                            psums[m][:],
                            lhsT=w_sb[:, m * P:(m + 1) * P],
                            rhs=xt_sb[k][:],
                            start=(k == 0),
                            stop=(k == KT - 1),
                        )
                for m in range(IT):
                    nc.any.tensor_copy(out=dst[:, m, :], in_=psums[m][:])

            # AllGather B across cores
            bloc = dram_pool.tile([P, IT, R], dt.bfloat16, tag="bloc")
            ball = dram_pool.tile([N_CORES, P, IT, R], dt.bfloat16, tag="ball")
            nc.sync.dma_start(out=bloc[:], in_=b_sb[:])
            nc.gpsimd.collective_compute(
                "AllGather",
                mybir.AluOpType.bypass,
                replica_groups=[list(range(N_CORES))],
                ins=[bloc.opt()],
                outs=[ball.opt()],
            )

            # Phase C: S rows = AT^T @ B, one 512-col block per gathered core
            for j in range(N_CORES):
                bj = bj_pool.tile([P, IT, R], dt.bfloat16, tag="bj")
                nc.sync.dma_start(out=bj[:], in_=ball[j])
                for m in range(R // P):  # 4 row m-tiles
                    ps = ps_pool.tile([P, R], dt.float32)
                    for k in range(IT):
                        nc.tensor.matmul(
                            ps[:],
                            lhsT=at_sb[:, k, m * P:(m + 1) * P],
                            rhs=bj[:, k, :],
                            start=(k == 0),
                            stop=(k == IT - 1),
                        )
                    ot = oc_pool.tile([P, R], dt.float32, tag="oc")
                    nc.any.tensor_copy(out=ot[:], in_=ps[:])
                    nc.sync.dma_start(out=out_ap[:, m, j * R:(j + 1) * R], in_=ot[:])

    nc.compile()
    return nc


def kernel(features: np.ndarray, P: np.ndarray, Q: np.ndarray) -> np.ndarray:
    from concourse.bass_utils import run_bass_kernel_spmd

    if "nc" not in _CACHE:
        _CACHE["nc"] = _build()
    nc = _CACHE["nc"]

    X = np.ascontiguousarray(features[0, 1:1 + D, :], dtype=np.float32)
    p_bf = np.ascontiguousarray(P, dtype=np.float32).astype(BF16)
    qt_bf = np.ascontiguousarray(Q.astype(np.float32).T).astype(BF16)

    in_maps = []
    for c in range(N_CORES):
        xt_c = np.ascontiguousarray(X[c * R:(c + 1) * R, :].T).astype(BF16)
        in_maps.append({"xt": xt_c, "p": p_bf, "qt": qt_bf})

    res = run_bass_kernel_spmd(nc, in_maps, list(range(N_CORES)))
    S = np.concatenate([res.results[c]["out"] for c in range(N_CORES)], axis=0)
    out = np.tril(S, k=-1).astype(np.float32)
    return out


# revision 6
# speedup vs baseline: 2.3352x; 2.3352x over previous
"""ContactMapLinear Trainium2 kernel.

res = tril((X @ P) @ (Q @ X^T), k=-1), X = features[0, 1:4097, :], 8-core SPMD.

Sharding (sequence-parallel): core c owns seq rows [512c, 512c+512).
  Phase A: AT_c = P^T @ X_c^T        [1024, 512]   (A = X@P, stored transposed)
  Phase B: B_c  = Q   @ X_c^T        [1024, 512]   (cols of B = Q @ X^T)
  AllGather B_c over 8 cores -> B    [1024, 4096]
  Phase C: S_c  = AT_c^T @ B         [512, 4096], masked on host (tril, k=-1)

All matmul inputs are bf16 (fp32 PSUM accumulation); output is fp32.
"""

import sys

import ml_dtypes
import numpy as np

_TRN_REPO = "/opt/trn_rl_repo"
if _TRN_REPO not in sys.path:
    sys.path.insert(0, _TRN_REPO)

D = 4096          # seq length / feature dim
I = 1024          # inner dim
N_CORES = 8
R = D // N_CORES  # 512 seq rows per core
P = 128           # partitions
KT = D // P       # 32 feature k-tiles
IT = I // P       # 8 inner tiles
BF16 = ml_dtypes.bfloat16

_CACHE = {}


def _build(repeat: int = 1):
    import concourse.mybir as mybir
    import concourse.tile as tile
    from concourse import bacc

    dt = mybir.dt
    nc = bacc.Bacc("TRN2", target_bir_lowering=False, debug=False,
                   num_devices=N_CORES)

    xt_in = nc.declare_dram_parameter("xt", [D, R], dt.bfloat16, isOutput=False)
    p_in = nc.declare_dram_parameter("p", [D, I], dt.bfloat16, isOutput=False)
    qt_in = nc.declare_dram_parameter("qt", [D, I], dt.bfloat16, isOutput=False)
    out = nc.declare_dram_parameter("out", [R, D], dt.float32, isOutput=True)

    xt_ap = xt_in.ap().rearrange("(ko ki) n -> ki ko n", ki=P)   # [128, 32, 512]
    p_ap = p_in.ap().rearrange("(ko ki) m -> ki ko m", ki=P)     # [128, 32, 1024]
    qt_ap = qt_in.ap().rearrange("(ko ki) m -> ki ko m", ki=P)
    out_ap = out.ap().rearrange("(mo mi) n -> mi mo n", mi=P)    # [128, 4, 4096]

    with tile.TileContext(nc) as tc:
        with (
            tc.tile_pool(name="xt", bufs=1) as xt_pool,
            tc.tile_pool(name="w", bufs=3) as w_pool,
            tc.tile_pool(name="ab", bufs=1) as ab_pool,
            tc.tile_pool(name="bj", bufs=2) as bj_pool,
            tc.tile_pool(name="oc", bufs=3) as oc_pool,
            tc.tile_pool(name="ps", bufs=1, space="PSUM") as ps_pool,
            tc.tile_pool(name="dram", bufs=1, space="DRAM") as dram_pool,
        ):
            for _rep in range(repeat):
                # xt tiles: one per feature k-tile so compute starts per-tile
                xt_sb = []
                for k in range(KT):
                    t = xt_pool.tile([P, R], dt.bfloat16, name=f"xt{k}",
                                     tag=f"xt{k}")
                    nc.sync.dma_start(out=t[:], in_=xt_ap[:, k, :])
                    xt_sb.append(t)

                at_sb = ab_pool.tile([P, IT, R], dt.bfloat16, name="at", tag="at")
                b_sb = ab_pool.tile([P, IT, R], dt.bfloat16, name="b", tag="b")

                # Phases A and B: k-outer streaming of weights, 8 PSUM banks
                # (one per inner m-tile) accumulate across all 32 k-tiles.
                for w_ap, dst in ((p_ap, at_sb), (qt_ap, b_sb)):
                    psums = [
                        ps_pool.tile([P, R], dt.float32, name=f"ps{m}",
                                     tag=f"ps{m}")
                        for m in range(IT)
                    ]
                    for k in range(KT):
                        w_sb = w_pool.tile([P, I], dt.bfloat16, name="w", tag="w")
                        nc.sync.dma_start(out=w_sb[:], in_=w_ap[:, k, :])
                        for m in range(IT):
                            nc.tensor.matmul(
                                psums[m][:],
                                lhsT=w_sb[:, m * P:(m + 1) * P],
                                rhs=xt_sb[k][:],
                                start=(k == 0),
                                stop=(k == KT - 1),
                            )
                    for m in range(IT):
                        nc.any.tensor_copy(out=dst[:, m, :], in_=psums[m][:])

                # AllGather B across cores
                bloc = dram_pool.tile([P, IT, R], dt.bfloat16, name="bloc",
                                      tag="bloc")
                ball = dram_pool.tile([N_CORES, P, IT, R], dt.bfloat16,
                                      name="ball", tag="ball",
                                      addr_space="Shared")
                nc.sync.dma_start(out=bloc[:], in_=b_sb[:])
                nc.gpsimd.collective_compute(
                    "AllGather",
                    mybir.AluOpType.bypass,
                    replica_groups=[list(range(N_CORES))],
                    ins=[bloc.opt()],
                    outs=[ball.opt()],
                )

                # Phase C: S rows = AT^T @ B, one 512-col block per core
                for j in range(N_CORES):
                    bj = bj_pool.tile([P, IT, R], dt.bfloat16, name="bj",
                                      tag="bj")
                    nc.sync.dma_start(out=bj[:], in_=ball[j])
                    for m in range(R // P):  # 4 row m-tiles
                        ps = ps_pool.tile([P, R], dt.float32, name=f"psc{m}",
                                          tag=f"ps{m}")
                        for k in range(IT):
                            nc.tensor.matmul(
                                ps[:],
                                lhsT=at_sb[:, k, m * P:(m + 1) * P],
                                rhs=bj[:, k, :],
                                start=(k == 0),
                                stop=(k == IT - 1),
                            )
                        ot = oc_pool.tile([P, R], dt.float32, name="oc",
                                          tag="oc")
                        nc.any.tensor_copy(out=ot[:], in_=ps[:])
                        nc.sync.dma_start(out=out_ap[:, m, j * R:(j + 1) * R],
                                          in_=ot[:])

    nc.compile()
    return nc


def _make_in_maps(features: np.ndarray, Pm: np.ndarray, Qm: np.ndarray):
    X = np.ascontiguousarray(features[0, 1:1 + D, :], dtype=np.float32)
    p_bf = np.ascontiguousarray(Pm, dtype=np.float32).astype(BF16)
    qt_bf = np.ascontiguousarray(Qm.astype(np.float32).T).astype(BF16)
    in_maps = []
    for c in range(N_CORES):
        xt_c = np.ascontiguousarray(X[c * R:(c + 1) * R, :].T).astype(BF16)
        in_maps.append({"xt": xt_c, "p": p_bf, "qt": qt_bf})
    return in_maps


def kernel(features: np.ndarray, P: np.ndarray, Q: np.ndarray) -> np.ndarray:
    from concourse.bass_utils import run_bass_kernel_spmd

    if "nc" not in _CACHE:
        _CACHE["nc"] = _build()
    nc = _CACHE["nc"]

    in_maps = _make_in_maps(features, P, Q)
    res = run_bass_kernel_spmd(nc, in_maps, list(range(N_CORES)))
    S = np.concatenate([res.results[c]["out"] for c in range(N_CORES)], axis=0)
    return np.tril(S, k=-1).astype(np.float32)
